# revision 21
# baseline (speedup 1.0000x reference)
"""Cross-attention layer kernel for Trainium2 (8 NeuronCores, data-parallel over batch).

Per-core computation (batch element b):
  Q_ = conv3(Q, wq@wd) ; K_ = conv3(K, wk@wd) ; V_ = conv3(V, wv@wd)   (conv1+conv3 fused)
  S^T = K_ @ Q_^T  (m on partitions, l on free dim)
  expS = exp(S^T)  (no max subtraction; |S| <~ 45 is safe in fp32)
  [U; rowsum] = [V_ | ones]^T @ expS   (softmax denominator fused into the AV matmul)
  out^T = U / rowsum ;  y^T = wo^T @ out^T + bo

The wall clock is dominated by the axon tunnel (~83 ms RTT, ~75-90 MB/s
H2D, ~50 MB/s D2H), so transport is quantized and packed into ONE u8
input blob + ONE u8 output blob (each extra array costs ~5 ms):
  Q/K: 12-bit per-channel quant, planar-packed 2 values / 3 bytes,
       unpacked on device with u8 shift/and + activation converts into
       exact integer-valued fp16 (the dequant scales are folded into the
       per-core fp16 projection weights, so no scale op and no fp16
       rounding of the activations);
  V:   int8 per-channel (scale likewise folded into wv);
  out: int8 with per-(channel, 512-group) scales computed on device.
Projection matmuls consume fp16 with fp32 PSUM accumulate; the attention
core (exp needs fp32 range, softmax normalization) stays fp32. Total
transport error ~1.15e-2 vs the 2e-2 gate (validated against a numpy
simulation). Dispatch is AOT-compiled once via fast_dispatch_compile (no
per-call retrace) and issues a single pipelined put+exec+get chain.
"""

import numpy as np
import jax
from jax.sharding import Mesh, PartitionSpec
from jax.experimental.shard_map import shard_map

import concourse.bass as bass
import concourse.tile as tile
import concourse.bass2jax as b2j
from concourse import bacc, mybir

B, L, C = 8, 4096, 64
NCORES = 8
G = 512            # l-group width (columns of S^T per pass)
NG = L // G        # 8 l-groups
MC = 128           # m-chunk height
NMC = L // MC      # 32 m-chunks
F32 = mybir.dt.float32
F16 = mybir.dt.float16
I8 = mybir.dt.int8
U8 = mybir.dt.uint8
EXP = mybir.ActivationFunctionType.Exp
COPY = mybir.ActivationFunctionType.Copy
NPAIR = (L + 2) // 2      # 12-bit packed pairs per channel row
NPB = 3 * NPAIR           # packed bytes per channel row

# staging group sizes (chunks per exp-activation); 3 banks + 3 banks + 2 U banks = 8 PSUM banks
GROUPS = [3] * 10 + [2]
assert sum(GROUPS) == NMC

# misc f32 blob columns: wo[0:64], bq, bk, bo, eq0, eq1, ek0, ek1, vscale
MISC_COLS = 72

# single u8 input blob layout (per-core [64, MCOLS], byte column offsets)
OFF_QP = 0                      # qp12 [64, NPB] u8
OFF_KP = OFF_QP + 3 * ((L + 2) // 2)
OFF_V8 = OFF_KP + 3 * ((L + 2) // 2)   # vt8 [64, L+2] i8
OFF_W16 = OFF_V8 + (L + 2)      # w16 [64, 576] f16
OFF_MISC = OFF_W16 + 2 * 576    # misc [64, MISC_COLS] f32
OFF_EXT = OFF_MISC + 4 * MISC_COLS  # rows 0/1: ev0/ev1 f32[64]; row 2: bv f32[64]
MCOLS = OFF_EXT + 256
# output blob: yt8 [64, L] i8 || ysc [64, NG] f32
OCOLS = L + 4 * NG


def build_program():
    nc = bacc.Bacc("TRN2", target_bir_lowering=False, debug=False, num_devices=NCORES)

    meg_d = nc.dram_tensor("meg", [C, MCOLS], U8, kind="ExternalInput")
    out_d = nc.dram_tensor("out", [C, OCOLS], U8, kind="ExternalOutput")

    def qp_ap():
        return meg_d[:, OFF_QP : OFF_QP + NPB]

    def kp_ap():
        return meg_d[:, OFF_KP : OFF_KP + NPB]

    def v8_ap():
        return meg_d[:, OFF_V8 : OFF_V8 + L + 2].bitcast(I8)

    def w16_ap(c0, w):
        return meg_d[:, OFF_W16 + 2 * c0 : OFF_W16 + 2 * (c0 + w)].bitcast(F16)

    def misc_ap(c0, w):
        return meg_d[:, OFF_MISC + 4 * c0 : OFF_MISC + 4 * (c0 + w)].bitcast(F32)

    def ext_ap(row):
        return meg_d[row : row + 1, OFF_EXT : OFF_EXT + 256].bitcast(F32)

    with tile.TileContext(nc) as tc:
        with tc.tile_pool(name="persist", bufs=1) as per:
            qin = per.tile([128, L + 2], F16)
            kin = per.tile([128, L + 2], F16)
            vin8 = per.tile([128, L + 2], I8)
            qp8 = per.tile([128, NPB], U8)
            kp8 = per.tile([128, NPB], U8)
            # duplicate channel-major inputs into both partition halves (row tiling)
            for dst, src in ((qp8, qp_ap()), (kp8, kp_ap()), (vin8, v8_ap())):
                nc.sync.dma_start(out=dst[0:C, :], in_=src)
                nc.sync.dma_start(out=dst[C : 2 * C, :], in_=src)

            # ---- unpack 12-bit planar (b0|b1|b2 planes) into exact
            # integer-valued f16 (dequant scale is folded into the per-core
            # projection weights): even = b0*16 + (b1>>4) - 2048,
            # odd = (b1&15)*256 + b2 - 2048
            A = mybir.AluOpType
            with tc.tile_pool(name="unp", bufs=2) as unp:
                for p8, xin in ((qp8, qin), (kp8, kin)):
                    hi8 = unp.tile([128, NPAIR], U8, tag="hi", name="hi8")
                    lo8 = unp.tile([128, NPAIR], U8, tag="lo", name="lo8")
                    b0s = unp.tile([128, NPAIR], F32, tag="b0", name="b0s")
                    hif = unp.tile([128, NPAIR], F32, tag="hf", name="hif")
                    los = unp.tile([128, NPAIR], F32, tag="lf", name="los")
                    b2f = unp.tile([128, NPAIR], F32, tag="b2", name="b2f")
                    b1 = p8[:, NPAIR : 2 * NPAIR]
                    nc.vector.tensor_scalar(
                        out=hi8, in0=b1, scalar1=4, scalar2=None,
                        op0=A.logical_shift_right,
                    )
                    nc.vector.tensor_scalar(
                        out=lo8, in0=b1, scalar1=15, scalar2=None, op0=A.bitwise_and
                    )
                    nc.scalar.activation(
                        out=b0s, in_=p8[:, 0:NPAIR], func=COPY, scale=16.0, bias=-2048.0
                    )
                    nc.scalar.activation(out=hif, in_=hi8, func=COPY)
                    nc.scalar.activation(
                        out=los, in_=lo8, func=COPY, scale=256.0, bias=-2048.0
                    )
                    nc.scalar.activation(
                        out=b2f, in_=p8[:, 2 * NPAIR : 3 * NPAIR], func=COPY
                    )
                    nc.vector.scalar_tensor_tensor(
                        out=xin[:, 0 : L + 2 : 2], in0=hif, scalar=0.0, in1=b0s,
                        op0=A.add, op1=A.add,
                    )
                    nc.vector.scalar_tensor_tensor(
                        out=xin[:, 1 : L + 2 : 2], in0=los, scalar=0.0, in1=b2f,
                        op0=A.add, op1=A.add,
                    )

            # weights: expand the compact [64, 3*64] blocks into the
            # duplicated-halves / duplicated-cols layouts the matmuls use
            wq_sb = per.tile([128, 3, 128], F16)
            wk_sb = per.tile([128, 3, 128], F16)
            for dst, c0 in ((wq_sb, 0), (wk_sb, 192)):
                for h in range(2):
                    for k in range(3):
                        for cc in range(2):
                            nc.sync.dma_start(
                                out=dst[64 * h : 64 * h + 64, k, 64 * cc : 64 * cc + 64],
                                in_=w16_ap(c0 + 64 * k, 64),
                            )
            wv_sb = per.tile([128, 3, C], F16)
            for h in range(2):
                for k in range(3):
                    nc.sync.dma_start(
                        out=wv_sb[64 * h : 64 * h + 64, k, :],
                        in_=w16_ap(384 + 64 * k, 64),
                    )
            wo_sb = per.tile([C, C], F32)
            nc.sync.dma_start(out=wo_sb, in_=misc_ap(0, 64))
            bq_sb = per.tile([128, 1], F32)
            bk_sb = per.tile([128, 1], F32)
            eq_sb = per.tile([128, 2], F32)
            ek_sb = per.tile([128, 2], F32)
            for dst, c0, w in ((bq_sb, 64, 1), (bk_sb, 65, 1), (eq_sb, 67, 2),
                               (ek_sb, 69, 2)):
                for h in range(2):
                    nc.sync.dma_start(
                        out=dst[64 * h : 64 * h + 64, :], in_=misc_ap(c0, w)
                    )
            bo_sb = per.tile([C, 1], F32)
            nc.sync.dma_start(out=bo_sb, in_=misc_ap(66, 1))
            bv_sb = per.tile([128, C], F32)
            nc.sync.dma_start(out=bv_sb, in_=ext_ap(2).to_broadcast((128, C)))
            ev_sb = per.tile([128, C], F32)
            nc.vector.memset(ev_sb[:, :], 0.0)
            nc.sync.dma_start(out=ev_sb[0:1, :], in_=ext_ap(0))
            nc.sync.dma_start(out=ev_sb[127:128, :], in_=ext_ap(1))

            # V int8 -> f16 (dequant scale is folded into wv)
            vin = per.tile([128, L + 2], F16)
            nc.scalar.activation(out=vin, in_=vin8, func=COPY)

            qT = per.tile([128, L], F32)   # Q_^T, duplicated halves
            kT = per.tile([128, L], F32)   # K_^T, duplicated halves
            vrow = per.tile([128, NMC, C + 1], F32)  # V_ row-major chunks + ones col

            # ---------------- projections ----------------
            with tc.tile_pool(name="pqk", bufs=4, space="PSUM") as pqk, tc.tile_pool(
                name="pv", bufs=4, space="PSUM"
            ) as pv:
                for xin, w_sb, b_sb, xT in (
                    (qin, wq_sb, bq_sb, qT),
                    (kin, wk_sb, bk_sb, kT),
                ):
                    for g0 in range(0, NG, 2):
                        psA = pqk.tile([128, G], F32, tag="qk", name="psA")
                        psB = pqk.tile([128, G], F32, tag="qk", name="psB")
                        for k in range(3):
                            nc.tensor.matmul(
                                psA,
                                lhsT=w_sb[0:64, k, :],
                                rhs=xin[0:64, g0 * G + k : g0 * G + k + G],
                                start=(k == 0),
                                stop=(k == 2),
                                tile_position=(0, 0),
                            )
                            nc.tensor.matmul(
                                psB,
                                lhsT=w_sb[64:128, k, :],
                                rhs=xin[64:128, (g0 + 1) * G + k : (g0 + 1) * G + k + G],
                                start=(k == 0),
                                stop=(k == 2),
                                tile_position=(64, 0),
                            )
                        nc.vector.tensor_scalar_add(
                            out=xT[:, g0 * G : (g0 + 1) * G], in0=psA, scalar1=b_sb
                        )
                        nc.vector.tensor_scalar_add(
                            out=xT[:, (g0 + 1) * G : (g0 + 2) * G], in0=psB, scalar1=b_sb
                        )
                # conv edge corrections (pad column saw folded conv1 bias)
                nc.vector.tensor_scalar_add(
                    out=qT[:, 0:1], in0=qT[:, 0:1], scalar1=eq_sb[:, 0:1]
                )
                nc.vector.tensor_scalar_add(
                    out=qT[:, L - 1 : L], in0=qT[:, L - 1 : L], scalar1=eq_sb[:, 1:2]
                )
                nc.vector.tensor_scalar_add(
                    out=kT[:, 0:1], in0=kT[:, 0:1], scalar1=ek_sb[:, 0:1]
                )
                nc.vector.tensor_scalar_add(
                    out=kT[:, L - 1 : L], in0=kT[:, L - 1 : L], scalar1=ek_sb[:, 1:2]
                )

                # V_ row-major conv (shifted-window lhsT), paired row tiles
                nc.vector.memset(vrow[:, :, C : C + 1], 1.0)
                for c0 in range(0, NMC, 2):
                    pvA = pv.tile([128, C], F32, tag="v", name="pvA")
                    pvB = pv.tile([128, C], F32, tag="v", name="pvB")
                    for k in range(3):
                        nc.tensor.matmul(
                            pvA,
                            lhsT=vin[0:64, c0 * MC + k : c0 * MC + k + MC],
                            rhs=wv_sb[0:64, k, :],
                            start=(k == 0),
                            stop=(k == 2),
                            tile_position=(0, 0),
                        )
                        nc.tensor.matmul(
                            pvB,
                            lhsT=vin[64:128, (c0 + 1) * MC + k : (c0 + 1) * MC + k + MC],
                            rhs=wv_sb[64:128, k, :],
                            start=(k == 0),
                            stop=(k == 2),
                            tile_position=(64, 0),
                        )
                    nc.vector.tensor_add(out=vrow[:, c0, 0:C], in0=pvA, in1=bv_sb)
                    nc.vector.tensor_add(out=vrow[:, c0 + 1, 0:C], in0=pvB, in1=bv_sb)
                nc.vector.tensor_add(
                    out=vrow[0:1, 0, 0:C], in0=vrow[0:1, 0, 0:C], in1=ev_sb[0:1, :]
                )
                nc.vector.tensor_add(
                    out=vrow[96:128, NMC - 1, 0:C],
                    in0=vrow[96:128, NMC - 1, 0:C],
                    in1=ev_sb[96:128, :],
                )

            # ---------------- attention ----------------
            with tc.tile_pool(name="stg", bufs=2, space="PSUM") as stg, tc.tile_pool(
                name="ups", bufs=1, space="PSUM"
            ) as ups, tc.tile_pool(name="esb", bufs=3) as esb, tc.tile_pool(
                name="osb", bufs=2
            ) as osb, tc.tile_pool(name="drp", bufs=2, space="DRAM") as drp:
                for g in range(NG):
                    ua = ups.tile([128, G], F32, tag="ua", name="ua")
                    ub = ups.tile([128, G], F32, tag="ub", name="ub")
                    qs_lo = qT[0:64, g * G : (g + 1) * G]
                    qs_hi = qT[64:128, g * G : (g + 1) * G]
                    prev = None
                    c = 0
                    for gs in GROUPS:
                        st = stg.tile([128, 3 * G], F32, tag="st", name="st")
                        for i in range(0, gs, 2):
                            ca = c + i
                            nc.tensor.matmul(
                                st[:, i * G : (i + 1) * G],
                                lhsT=kT[0:64, ca * MC : (ca + 1) * MC],
                                rhs=qs_lo,
                                start=True,
                                stop=True,
                                tile_position=(0, 0),
                            )
                            if i + 1 < gs:
                                cb = c + i + 1
                                nc.tensor.matmul(
                                    st[:, (i + 1) * G : (i + 2) * G],
                                    lhsT=kT[64:128, cb * MC : (cb + 1) * MC],
                                    rhs=qs_hi,
                                    start=True,
                                    stop=True,
                                    tile_position=(64, 0),
                                )
                        es = esb.tile([128, 3 * G], F32, tag="es", name="es")
                        nc.scalar.activation(
                            out=es[:, : gs * G], in_=st[:, : gs * G], func=EXP
                        )
                        if prev is not None:
                            pes, pc, pgs = prev
                            for i in range(pgs):
                                cc = pc + i
                                nc.tensor.matmul(
                                    ua[0:65, :],
                                    lhsT=vrow[0:64, cc, :],
                                    rhs=pes[0:64, i * G : (i + 1) * G],
                                    start=(cc == 0),
                                    stop=False,
                                    tile_position=(0, 0),
                                )
                                nc.tensor.matmul(
                                    ub[0:65, :],
                                    lhsT=vrow[64:128, cc, :],
                                    rhs=pes[64:128, i * G : (i + 1) * G],
                                    start=(cc == 0),
                                    stop=False,
                                    tile_position=(64, 0),
                                )
                        prev = (es, c, gs)
                        c += gs
                    pes, pc, pgs = prev
                    for i in range(pgs):
                        cc = pc + i
                        nc.tensor.matmul(
                            ua[0:65, :],
                            lhsT=vrow[0:64, cc, :],
                            rhs=pes[0:64, i * G : (i + 1) * G],
                            start=False,
                            stop=(cc == NMC - 1),
                            tile_position=(0, 0),
                        )
                        nc.tensor.matmul(
                            ub[0:65, :],
                            lhsT=vrow[64:128, cc, :],
                            rhs=pes[64:128, i * G : (i + 1) * G],
                            start=False,
                            stop=(cc == NMC - 1),
                            tile_position=(64, 0),
                        )

                    # normalize: usum = ua + ub ; out^T = usum[:64] / usum[64]
                    # (DVE may read only one PSUM operand per instruction)
                    ubs = osb.tile([65, G], F32, tag="ubs", name="ubs")
                    nc.vector.tensor_copy(out=ubs, in_=ub[0:65, :])
                    usum = osb.tile([65, G], F32, tag="us", name="usum")
                    nc.vector.tensor_add(out=usum, in0=ua[0:65, :], in1=ubs)
                    rec = osb.tile([65, G], F32, tag="rc", name="rec")
                    nc.vector.reciprocal(out=rec[64:65, :], in_=usum[64:65, :])
                    # partition-broadcast via DRAM bounce (custom GPSIMD bcast
                    # ucode does not honor the partition-64 source AP on HW)
                    rb = drp.tile([1, G], F32, tag="rb", name="rb")
                    nc.sync.dma_start(out=rb, in_=rec[64:65, :])
                    r64 = osb.tile([64, G], F32, tag="r64", name="r64")
                    nc.sync.dma_start(out=r64, in_=rb[:, :].to_broadcast((64, G)))
                    outT = osb.tile([64, G], F32, tag="ot", name="outT")
                    nc.vector.tensor_mul(out=outT, in0=usum[0:64, :], in1=r64)

                    # output projection: y^T = wo^T @ out^T + bo
                    yp = ups.tile([128, G], F32, tag="ua", name="yp")
                    nc.tensor.matmul(
                        yp[0:64, :],
                        lhsT=wo_sb,
                        rhs=outT,
                        start=True,
                        stop=True,
                        tile_position=(0, 0),
                    )
                    # int8 output with per-(channel, group) scale
                    ysb = osb.tile([64, G], F32, tag="y", name="ysb")
                    nc.vector.tensor_scalar_add(out=ysb, in0=yp[0:64, :], scalar1=bo_sb)
                    amax = osb.tile([64, 4], F32, tag="am", name="amax")
                    nc.vector.tensor_reduce(
                        out=amax[:, 0:1], in_=ysb, axis=mybir.AxisListType.X,
                        op=mybir.AluOpType.max, apply_absolute_value=True,
                    )
                    nc.vector.tensor_scalar_max(
                        out=amax[:, 1:2], in0=amax[:, 0:1], scalar1=1e-30
                    )
                    nc.vector.reciprocal(out=amax[:, 2:3], in_=amax[:, 1:2])
                    nc.vector.tensor_scalar_mul(
                        out=amax[:, 3:4], in0=amax[:, 2:3], scalar1=127.0
                    )
                    y8 = osb.tile([64, G], I8, tag="y8", name="y8")
                    nc.scalar.activation(
                        out=y8, in_=ysb, func=COPY, scale=amax[:, 3:4]
                    )
                    ysc = osb.tile([64, 1], F32, tag="ys", name="ysc")
                    nc.vector.tensor_scalar_mul(
                        out=ysc, in0=amax[:, 1:2], scalar1=1.0 / 127.0
                    )
                    nc.sync.dma_start(
                        out=out_d[:, g * G : (g + 1) * G].bitcast(I8), in_=y8
                    )
                    nc.sync.dma_start(
                        out=out_d[:, L + 4 * g : L + 4 * (g + 1)].bitcast(F32),
                        in_=ysc,
                    )

    nc.compile()
    return nc


_STATE = None


def _get_state():
    """Build the Bass program and AOT-compile the 8-core dispatch once."""
    global _STATE
    if _STATE is not None:
        return _STATE
    b2j.install_neuronx_cc_hook()
    nc = build_program()
    partition_name = nc.partition_id_tensor.name if nc.partition_id_tensor else None
    in_names, in_shapes, out_names, out_avals = [], [], [], []
    for alloc in nc.m.functions[0].allocations:
        if not isinstance(alloc, mybir.MemoryLocationSet):
            continue
        name = alloc.memorylocations[0].name
        if alloc.kind == "ExternalInput":
            if name != partition_name:
                in_names.append(name)
                in_shapes.append(
                    (tuple(alloc.tensor_shape), mybir.dt.np(alloc.dtype))
                )
        elif alloc.kind == "ExternalOutput":
            out_names.append(name)
            out_avals.append(
                jax.core.ShapedArray(
                    tuple(alloc.tensor_shape), mybir.dt.np(alloc.dtype)
                )
            )

    bind_in_names = tuple(in_names) + ((partition_name,) if partition_name else ())

    def _body(*args):
        operands = list(args)
        if partition_name is not None:
            operands.append(b2j.partition_id_tensor())
        return tuple(
            b2j._bass_exec_p.bind(
                *operands,
                out_avals=tuple(out_avals),
                in_names=bind_in_names,
                out_names=tuple(out_names),
                lowering_input_output_aliases=(),
                sim_require_finite=True,
                sim_require_nnan=True,
                nc=nc,
            )
        )

    devices = jax.devices()[:NCORES]
    mesh = Mesh(np.asarray(devices), ("core",))
    fn = shard_map(
        _body,
        mesh=mesh,
        in_specs=(PartitionSpec("core"),) * len(in_names),
        out_specs=(PartitionSpec("core"),) * len(out_names),
        check_rep=False,
    )
    global_avals = [
        jax.ShapeDtypeStruct((NCORES * s[0][0],) + tuple(s[0][1:]), s[1])
        for s in in_shapes
    ]
    compiled = b2j.fast_dispatch_compile(
        lambda: jax.jit(fn).lower(*global_avals).compile()
    )
    _STATE = (compiled, in_names, out_names)
    return _STATE


def make_in_maps(Q, K, V, wq, bq, wk, bk, wv, bv, wd, bd, wo, bo):
    """Fold weights, quantize transport, build the global (8*64, ...) arrays."""
    f32 = np.float32

    def fold(w1):
        return np.stack([w1[0].astype(f32) @ wd[k].astype(f32) for k in range(3)], 0)

    wqd, wkd, wvd = fold(wq), fold(wk), fold(wv)
    sum_wd = (wd[0] + wd[1] + wd[2]).astype(f32)

    def fold_bias(b1):
        return (b1.astype(f32) @ sum_wd + bd.astype(f32)).astype(f32)

    bqd, bkd, bvd = fold_bias(bq), fold_bias(bk), fold_bias(bv)

    def edges(b1):
        e0 = -(b1.astype(f32) @ wd[0].astype(f32))
        e1 = -(b1.astype(f32) @ wd[2].astype(f32))
        return e0.astype(f32), e1.astype(f32)

    eq0, eq1 = edges(bq)
    ek0, ek1 = edges(bk)
    ev0, ev1 = edges(bv)

    misc = np.zeros((C, MISC_COLS), f32)
    misc[:, 0:64] = wo[0].astype(f32)
    misc[:, 64] = bqd
    misc[:, 65] = bkd
    misc[:, 66] = bo.astype(f32)
    misc[:, 67] = eq0
    misc[:, 68] = eq1
    misc[:, 69] = ek0
    misc[:, 70] = ek1

    ev = np.stack([ev0, ev1], 0).astype(f32)
    bv1 = bvd.reshape(1, C)

    def pack12(xt):
        # xt: [C, L+2] f32 (pad cols 0). Returns planar b0|b1|b2 bytes + scale.
        s = np.abs(xt).max(axis=1) / 2047.0 + 1e-30
        u = (
            np.clip(np.rint(xt / s[:, None]), -2047, 2047).astype(np.int32) + 2048
        )
        u0, u1 = u[:, 0::2], u[:, 1::2]
        p = np.empty((C, NPB), np.uint8)
        p[:, 0:NPAIR] = u0 >> 4
        p[:, NPAIR : 2 * NPAIR] = ((u0 & 15) << 4) | (u1 >> 8)
        p[:, 2 * NPAIR :] = u1 & 255
        return p, s

    meg = np.zeros((B * C, MCOLS), np.uint8)
    pad = np.zeros((C, 1), f32)
    misc_u8 = np.ascontiguousarray(misc).view(np.uint8)
    for b in range(B):
        r = slice(b * C, (b + 1) * C)
        qt = np.concatenate([pad, Q[b].astype(f32).T, pad], axis=1)
        kt = np.concatenate([pad, K[b].astype(f32).T, pad], axis=1)
        meg[r, OFF_QP : OFF_QP + NPB], sq = pack12(qt)
        meg[r, OFF_KP : OFF_KP + NPB], sk = pack12(kt)
        vt = np.ascontiguousarray(V[b].astype(f32).T)
        sv = np.abs(vt).max(axis=1) / 127.0 + 1e-30
        meg[r, OFF_V8 + 1 : OFF_V8 + L + 1] = (
            np.clip(np.rint(vt / sv[:, None]), -127, 127).astype(np.int8)
        ).view(np.uint8)
        # dequant scales fold into the per-core projection weights (row = in-ch)
        w16 = np.empty((C, 576), np.float16)
        for k in range(3):
            w16[:, 64 * k : 64 * k + 64] = wqd[k] * sq[:, None]
            w16[:, 192 + 64 * k : 192 + 64 * k + 64] = wkd[k] * sk[:, None]
            w16[:, 384 + 64 * k : 384 + 64 * k + 64] = wvd[k] * sv[:, None]
        meg[r, OFF_W16 : OFF_W16 + 1152] = w16.view(np.uint8)
        meg[r, OFF_MISC : OFF_MISC + 4 * MISC_COLS] = misc_u8
        meg[b * C + 0, OFF_EXT : OFF_EXT + 256] = (
            np.ascontiguousarray(ev[0]).view(np.uint8)
        )
        meg[b * C + 1, OFF_EXT : OFF_EXT + 256] = (
            np.ascontiguousarray(ev[1]).view(np.uint8)
        )
        meg[b * C + 2, OFF_EXT : OFF_EXT + 256] = (
            np.ascontiguousarray(bv1[0]).view(np.uint8)
        )
    return {"meg": meg}


def run_dispatch(gmap):
    """Single pipelined put+exec+get chain; AOT executable is cached."""
    compiled, in_names, out_names = _get_state()
    outs = compiled(*[gmap[n] for n in in_names])
    return dict(zip(out_names, jax.device_get(list(outs))))


def kernel(**inputs):
    gmap = make_in_maps(
        np.asarray(inputs["Q"], np.float32),
        np.asarray(inputs["K"], np.float32),
        np.asarray(inputs["V"], np.float32),
        np.asarray(inputs["wq"], np.float32), np.asarray(inputs["bq"], np.float32),
        np.asarray(inputs["wk"], np.float32), np.asarray(inputs["bk"], np.float32),
        np.asarray(inputs["wv"], np.float32), np.asarray(inputs["bv"], np.float32),
        np.asarray(inputs["wd"], np.float32), np.asarray(inputs["bd"], np.float32),
        np.asarray(inputs["wo"], np.float32), np.asarray(inputs["bo"], np.float32),
    )
    res = run_dispatch(gmap)
    blob = res["out"]  # [8*64, OCOLS] u8: yt8 i8 || ysc f32
    yt8 = blob[:, 0:L].view(np.int8)
    ysc = blob[:, L:].view(np.float32)  # [8*64, NG] per-(channel, group) scales
    y = yt8.astype(np.float32).reshape(B * C, NG, G) * ysc[:, :, None]
    y = y.reshape(B * C, L)
    out = np.empty((B, L, C), np.float32)
    for b in range(B):
        out[b] = y[b * C : (b + 1) * C].T
    return out
